# revision 6
# baseline (speedup 1.0000x reference)
"""ChannelWiseFloat8GroupedLinear — expert-parallel Trainium2 Bass kernel.

Problem: x [8192, 1024] f32, weight [8*1024, 1024] f32, tokens_per_expert
[8] int32 (uniform 1024).  out[t, d] = x_dq @ w_dq[e(t)].T in bf16, where
x is fp8-e4m3fn quant-dequantized per token row and w per expert block.

Sharding: expert-parallel over 8 NeuronCores.  Tokens are contiguous per
expert (cumsum offsets), so core e owns x rows [1024e, 1024e+1024) and
expert e's weight block — no cross-core communication.  The weight block
is fed to the device pre-transposed ([din, dout], standard
weights-stored-transposed layout) so the contraction dim lands on SBUF
partitions without PE transpose passes.

Device math: the reference quantizes to OCP e4m3fn (max 448); TRN2's
fp8_e4m3 tops out at 240.  Quantizing with r = 224/amax instead of
448/amax lands on the halved e4m3fn grid, which TRN e4m3 represents
exactly, and the x4 is folded into the output scale
m[t] = amax_x[t]*amax_w*4/448^2.  The fp8 matmuls run in DoubleRow perf
mode (two 128-deep contraction tiles per instruction, 2x rate), with
exact f32 PSUM accumulation.  x tiles are transposed on the PE two at a
time: one DoubleRow matmul against a [I|0 / 0|I] fp8 constant yields
[xA^T | xB^T] in a single pass.
"""

import numpy as np
import ml_dtypes

P = 128
TPE = 1024   # tokens per expert (= T // ne, uniform)
DIN = 1024
DOUT = 1024
NE = 8
NT = TPE // P    # 8 token tiles per core
NK = DIN // P    # 8 contraction tiles
E4M3_MAX = 448.0
EPS = 1e-12

_CACHE = {}


def _axon_device_reset():
    """Best-effort reset of the axon-tunneled NeuronCores after an
    NRT_EXEC_UNIT_UNRECOVERABLE wedge (observed rarely; a reset recovers)."""
    try:
        import ctypes

        import jax

        jax.devices()
        lib = ctypes.CDLL("/opt/axon/libaxon_pjrt.so")
        if hasattr(lib, "axon_reset"):
            lib.axon_reset.restype = ctypes.c_int64
            lib.axon_reset()
    except Exception:
        pass


def _build_nc():
    """Build + compile the single-core Bass program (run SPMD on 8 cores)."""
    import concourse.mybir as mybir
    import concourse.tile as tile
    from concourse import bacc, bass_isa
    from concourse.masks import make_identity

    dt = mybir.dt
    X = mybir.AxisListType.X
    ALU = mybir.AluOpType
    DR = mybir.MatmulPerfMode.DoubleRow

    nc = bacc.Bacc("TRN2", target_bir_lowering=False, debug=False)
    x_t = nc.dram_tensor("x", [TPE, DIN], dt.float32, kind="ExternalInput")
    w_t = nc.dram_tensor("wt", [DIN, DOUT], dt.float32, kind="ExternalInput")
    o_t = nc.dram_tensor("o", [TPE, DOUT], dt.bfloat16, kind="ExternalOutput")

    x_d = x_t.ap().rearrange("(tt p) k -> p tt k", p=P)    # [128, 8, 1024]
    w_d = w_t.ap().rearrange("(kk p) d -> p kk d", p=P)    # [128, 8, 1024]
    o_d = o_t.ap().rearrange("(tt p) d -> p tt d", p=P)

    with tile.TileContext(nc) as tc:
        with (
            tc.tile_pool(name="const", bufs=1) as const,
            tc.tile_pool(name="big", bufs=1) as big,
            tc.tile_pool(name="small", bufs=1) as small,
            tc.tile_pool(name="outp", bufs=3) as outp,
            tc.tile_pool(name="pt", bufs=2, space="PSUM") as pt,
            tc.tile_pool(name="pm", bufs=2, space="PSUM") as pm,
        ):
            # persistent buffers
            w_sb = big.tile([P, NK, DOUT], dt.float32, tag="w_sb")   # wT, k on partitions
            x_sb = big.tile([P, NT, DIN], dt.float32, tag="x_sb")
            qwT = big.tile([P, NK, DOUT], dt.float8e4, tag="qwT")
            qx = big.tile([P, NT, DIN], dt.float8e4, tag="qx")
            qxT = big.tile([P, NT, NK, P], dt.float8e4, tag="qxT")

            amw_parts = small.tile([P, NK], dt.float32, tag="amw_parts")
            amw_c = small.tile([P, 1], dt.float32, tag="amw_c")
            amw_g = small.tile([P, 1], dt.float32, tag="amw_g")
            inv_w = small.tile([P, 1], dt.float32, tag="inv_w")
            rw = small.tile([P, 1], dt.float32, tag="rw")
            cw = small.tile([P, 1], dt.float32, tag="cw")
            amx_parts = small.tile([P, NT], dt.float32, tag="amx_parts")
            amx_cl = small.tile([P, NT], dt.float32, tag="amx_cl")
            inv_x = small.tile([P, NT], dt.float32, tag="inv_x")
            rx = small.tile([P, NT], dt.float32, tag="rx")
            m_all = small.tile([P, NT], dt.float32, tag="m_all")

            # --- loads: w first (its global amax gates the whole w pipeline),
            # then x. 0.5MB per dma_start keeps per-tile completion receipts. ---
            for i in range(NK):
                nc.sync.dma_start(w_sb[:, i, :], w_d[:, i, :])
            for i in range(NT):
                nc.sync.dma_start(x_sb[:, i, :], x_d[:, i, :])

            # paired-transpose constant: idp[:, 0, 0:128] = I, idp[:, 1, 128:256] = I
            # (fp8: 1.0 is exactly representable).  One DoubleRow matmul with
            # lhsT = [xA | xB] then computes xA^T@[I|0] + xB^T@[0|I] = [xA^T | xB^T].
            id_f32 = const.tile([P, P], dt.float32, tag="id32f")
            make_identity(nc, id_f32[:])
            idp = const.tile([P, 2, 2 * P], dt.float8e4, tag="idp")
            nc.gpsimd.memset(idp[:], 0)
            nc.vector.tensor_copy(idp[:, 0, 0:P], id_f32[:])
            nc.vector.tensor_copy(idp[:, 1, P : 2 * P], id_f32[:])

            # --- w amax partials on vector, firing as each w tile lands ---
            for kk in range(NK):
                nc.vector.reduce_max(
                    amw_parts[:, kk : kk + 1],
                    w_sb[:, kk, :],
                    axis=X,
                    apply_absolute_value=True,
                )
            nc.vector.reduce_max(amw_c[:], amw_parts[:], axis=X)
            nc.vector.tensor_scalar_max(amw_c[:], amw_c[:], EPS)
            nc.gpsimd.partition_all_reduce(
                amw_g[:], amw_c[:], channels=P, reduce_op=bass_isa.ReduceOp.max
            )
            nc.vector.reciprocal(inv_w[:], amw_g[:])
            nc.vector.tensor_scalar_mul(rw[:], inv_w[:], E4M3_MAX / 2.0)
            nc.vector.tensor_scalar_mul(cw[:], amw_g[:], 4.0 / (E4M3_MAX * E4M3_MAX))

            # --- w quantize straight from the (pre-transposed) f32 tiles.
            # Split across engines so the first kk pairs are ready the moment
            # the main matmul wants them; emission order = readiness order. ---
            nc.vector.tensor_scalar_mul(qwT[:, 0, :], w_sb[:, 0, :], rw[:])
            nc.vector.tensor_scalar_mul(qwT[:, 1, :], w_sb[:, 1, :], rw[:])
            nc.scalar.mul(qwT[:, 2, :], w_sb[:, 2, :], rw[:])
            nc.scalar.mul(qwT[:, 3, :], w_sb[:, 3, :], rw[:])
            nc.scalar.mul(qwT[:, 4, :], w_sb[:, 4, :], rw[:])

            def emit_x_chain(tt):
                sl = slice(tt, tt + 1)
                # amax + the small scale chain on vector (free-axis reduces
                # are vector-only); the 1M-element quant goes to gpsimd
                nc.vector.reduce_max(
                    amx_parts[:, sl],
                    x_sb[:, tt, :],
                    axis=X,
                    apply_absolute_value=True,
                )
                nc.vector.tensor_scalar_max(amx_cl[:, sl], amx_parts[:, sl], EPS)
                nc.vector.reciprocal(inv_x[:, sl], amx_cl[:, sl])
                nc.vector.tensor_scalar_mul(rx[:, sl], inv_x[:, sl], E4M3_MAX / 2.0)
                nc.gpsimd.tensor_scalar_mul(
                    qx[:, tt, :], x_sb[:, tt, :], rx[:, sl]
                )
                # paired PE transposes: jp covers kk = (2jp, 2jp+1)
                pxf = pt.tile([P, NK // 2, 2 * P], dt.float32, tag="pt")
                for jp in range(NK // 2):
                    lhsT = qx[:, tt, 2 * P * jp : 2 * P * (jp + 1)].rearrange(
                        "p (two f) -> p two f", two=2
                    )
                    nc.tensor.matmul(
                        pxf[:, jp, :], lhsT=lhsT, rhs=idp[:],
                        start=True, stop=True, perf_mode=DR,
                    )
                # evict PSUM -> qxT (contiguous [NK, P] per tt); gpsimd
                # cannot read PSUM, so alternate the two engines that can
                if tt % 2 == 0:
                    nc.scalar.copy(qxT[:, tt, :, :], pxf[:])
                else:
                    nc.vector.tensor_copy(qxT[:, tt, :, :], pxf[:])

            def emit_main(tt):
                po = pm.tile([P, DOUT], dt.float32, tag="pm")
                for j in range(NK // 2):
                    st, sp = j == 0, j == NK // 2 - 1
                    for h in range(2):
                        nc.tensor.matmul(
                            po[:, h * 512 : (h + 1) * 512],
                            lhsT=qxT[:, tt, 2 * j : 2 * j + 2, :],
                            rhs=qwT[:, 2 * j : 2 * j + 2, h * 512 : (h + 1) * 512],
                            start=st, stop=sp, perf_mode=DR,
                        )
                sl = slice(tt, tt + 1)
                nc.vector.tensor_scalar(
                    m_all[:, sl], amx_cl[:, sl], cw[:], None, op0=ALU.mult
                )
                # scale+store in dout halves on both PSUM-capable engines:
                # halves the serial latency on the critical last tile
                ob = outp.tile([P, DOUT], dt.bfloat16, tag="ob")
                nc.vector.tensor_scalar_mul(ob[:, 0:512], po[:, 0:512], m_all[:, sl])
                nc.scalar.mul(ob[:, 512:1024], po[:, 512:1024], m_all[:, sl])
                nc.sync.dma_start(o_d[:, tt, :], ob[:])

            # remaining qwT tiles: last pairs, needed latest by the main sweep
            nc.gpsimd.tensor_scalar_mul(qwT[:, 5, :], w_sb[:, 5, :], rw[:])
            nc.scalar.mul(qwT[:, 6, :], w_sb[:, 6, :], rw[:])
            nc.scalar.mul(qwT[:, 7, :], w_sb[:, 7, :], rw[:])

            # --- per-token-tile pipeline, PE one tile ahead of the main
            # matmul so transposes never sit behind a stalled matmul ---
            emit_x_chain(0)
            for tt in range(1, NT):
                emit_x_chain(tt)
                emit_main(tt - 1)
            emit_main(NT - 1)

    nc.compile()
    return nc


def get_nc():
    if "nc" not in _CACHE:
        _CACHE["nc"] = _build_nc()
    return _CACHE["nc"]


def make_in_maps(x, weight):
    x = np.ascontiguousarray(np.asarray(x, dtype=np.float32))
    w = np.ascontiguousarray(np.asarray(weight, dtype=np.float32))
    return [
        {
            "x": x[TPE * e : TPE * (e + 1)],
            "wt": np.ascontiguousarray(w[DOUT * e : DOUT * (e + 1)].T),
        }
        for e in range(NE)
    ]


def _host_reference(x, weight, tokens_per_expert):
    """Exact numpy port of the reference — fallback for non-uniform routing."""
    x = np.asarray(x, dtype=np.float32)
    w = np.asarray(weight, dtype=np.float32)
    tpe = np.asarray(tokens_per_expert, dtype=np.int64)
    ne = tpe.shape[0]
    T, din = x.shape
    dout = w.shape[0] // ne
    wr = w.reshape(ne, dout, din)

    def qd(v, axis, fmax):
        amax = np.max(np.abs(v), axis=axis, keepdims=True)
        scale = np.maximum(amax, EPS) / fmax
        q = np.clip(v / scale, -fmax, fmax).astype(ml_dtypes.float8_e4m3fn)
        return q.astype(np.float32) * scale

    w_dq = qd(wr, (1, 2), E4M3_MAX)
    x_dq = qd(x, -1, E4M3_MAX)
    offs = np.cumsum(tpe)
    starts = offs - tpe
    out = np.zeros((T, dout), np.float32)
    for e in range(ne):
        s, t = int(starts[e]), int(offs[e])
        if t > s:
            out[s:t] = x_dq[s:t] @ w_dq[e].T
    return out.astype(ml_dtypes.bfloat16)


def kernel(x, weight, tokens_per_expert):
    x = np.asarray(x)
    weight = np.asarray(weight)
    tpe = np.asarray(tokens_per_expert)
    uniform = (
        x.shape == (NE * TPE, DIN)
        and weight.shape == (NE * DOUT, DIN)
        and tpe.shape == (NE,)
        and bool(np.all(tpe.astype(np.int64) == TPE))
    )
    if not uniform:
        return _host_reference(x, weight, tpe)

    from concourse.bass_utils import run_bass_kernel_spmd

    nc = get_nc()
    in_maps = make_in_maps(x, weight)
    try:
        res = run_bass_kernel_spmd(nc, in_maps, core_ids=list(range(NE)))
    except Exception:
        # rare device wedge (NRT_EXEC_UNIT_UNRECOVERABLE) — reset and retry
        _axon_device_reset()
        res = run_bass_kernel_spmd(nc, in_maps, core_ids=list(range(NE)))
    return np.concatenate([res.results[e]["o"] for e in range(NE)], axis=0)


if __name__ == "__main__":
    rng = np.random.default_rng(0)
    x = rng.standard_normal((NE * TPE, DIN), dtype=np.float32)
    w = (rng.standard_normal((NE * DOUT, DIN), dtype=np.float32) * 0.02).astype(
        np.float32
    )
    tpe = np.full((NE,), TPE, dtype=np.int32)
    out = kernel(x, w, tpe)
    exp = _host_reference(x, w, tpe)
    a = out.astype(np.float64)
    b = exp.astype(np.float64)
    denom = max(np.abs(b).max(), 1e-30)
    print("absmax rel err:", np.abs(a - b).max() / denom)
    rms = np.sqrt(((a - b) ** 2).mean()) / np.sqrt((b**2).mean())
    print("rms rel err:", rms)


# revision 12
# speedup vs baseline: 2.8685x; 2.8685x over previous
"""ChannelWiseFloat8GroupedLinear — expert-parallel Trainium2 Bass kernel.

Problem: x [8192, 1024] f32, weight [8*1024, 1024] f32, tokens_per_expert
[8] int32 (uniform 1024).  out[t, d] = x_dq @ w_dq[e(t)].T in bf16, where
x is fp8-e4m3fn quant-dequantized per token row and w per expert block.

Sharding: expert-parallel over 8 NeuronCores.  Tokens are contiguous per
expert (cumsum offsets), so core e owns x rows [1024e, 1024e+1024) and
expert e's weight block — no cross-core communication.  The weight block
is fed to the device pre-transposed ([din, dout], standard
weights-stored-transposed layout) so the contraction dim lands on SBUF
partitions without PE transpose passes.

Device math: the reference quantizes to OCP e4m3fn (max 448); TRN2's
fp8_e4m3 tops out at 240.  Quantizing with r = 224/amax instead of
448/amax lands on the halved e4m3fn grid, which TRN e4m3 represents
exactly, and the x4 is folded into the output scale
m[t] = amax_x[t]*amax_w*4/448^2.  The fp8 matmuls run in DoubleRow perf
mode (two 128-deep contraction tiles per instruction, 2x rate), with
exact f32 PSUM accumulation.  x tiles are transposed on the PE two at a
time: one DoubleRow matmul against a [I|0 / 0|I] fp8 constant yields
[xA^T | xB^T] in a single pass.
"""

import numpy as np
import ml_dtypes

P = 128
TPE = 1024   # tokens per expert (= T // ne, uniform)
DIN = 1024
DOUT = 1024
NE = 8
NT = TPE // P    # 8 token tiles per core
NK = DIN // P    # 8 contraction tiles
E4M3_MAX = 448.0
EPS = 1e-12

_CACHE = {}


def _axon_device_reset():
    """Best-effort reset of the axon-tunneled NeuronCores after an
    NRT_EXEC_UNIT_UNRECOVERABLE wedge (observed rarely; a reset recovers)."""
    try:
        import ctypes

        import jax

        jax.devices()
        lib = ctypes.CDLL("/opt/axon/libaxon_pjrt.so")
        if hasattr(lib, "axon_reset"):
            lib.axon_reset.restype = ctypes.c_int64
            lib.axon_reset()
    except Exception:
        pass


def _build_nc():
    """Build + compile the single-core Bass program (run SPMD on 8 cores)."""
    import concourse.mybir as mybir
    import concourse.tile as tile
    from concourse import bacc, bass_isa
    from concourse.masks import make_identity

    dt = mybir.dt
    X = mybir.AxisListType.X
    ALU = mybir.AluOpType
    DR = mybir.MatmulPerfMode.DoubleRow

    nc = bacc.Bacc("TRN2", target_bir_lowering=False, debug=False)
    x_t = nc.dram_tensor("x", [TPE, DIN], dt.float32, kind="ExternalInput")
    w_t = nc.dram_tensor("wt", [DIN, DOUT], dt.float32, kind="ExternalInput")
    o_t = nc.dram_tensor("o", [TPE, DOUT], dt.bfloat16, kind="ExternalOutput")

    x_d = x_t.ap().rearrange("(tt p) k -> p tt k", p=P)    # [128, 8, 1024]
    w_d = w_t.ap().rearrange("(kk p) d -> p kk d", p=P)    # [128, 8, 1024]
    o_d = o_t.ap().rearrange("(tt p) d -> p tt d", p=P)

    with tile.TileContext(nc) as tc:
        with (
            tc.tile_pool(name="const", bufs=1) as const,
            tc.tile_pool(name="big", bufs=1) as big,
            tc.tile_pool(name="small", bufs=1) as small,
            tc.tile_pool(name="outp", bufs=3) as outp,
            tc.tile_pool(name="pt", bufs=2, space="PSUM") as pt,
            tc.tile_pool(name="pm", bufs=2, space="PSUM") as pm,
        ):
            # persistent buffers
            w_sb = big.tile([P, NK, DOUT], dt.float32, tag="w_sb")   # wT, k on partitions
            x_sb = big.tile([P, NT, DIN], dt.float32, tag="x_sb")
            qwT = big.tile([P, NK, DOUT], dt.float8e4, tag="qwT")
            qx = big.tile([P, NT, DIN], dt.float8e4, tag="qx")
            qxT = big.tile([P, NT, NK, P], dt.float8e4, tag="qxT")

            amw_parts = small.tile([P, NK + 1], dt.float32, tag="amw_parts")
            amw_c = small.tile([P, 1], dt.float32, tag="amw_c")
            amw_g = small.tile([P, 1], dt.float32, tag="amw_g")
            inv_w = small.tile([P, 1], dt.float32, tag="inv_w")
            rw = small.tile([P, 1], dt.float32, tag="rw")
            cw = small.tile([P, 1], dt.float32, tag="cw")
            amx_parts = small.tile([P, NT], dt.float32, tag="amx_parts")
            amx_cl = small.tile([P, NT], dt.float32, tag="amx_cl")
            inv_x = small.tile([P, NT], dt.float32, tag="inv_x")
            rx = small.tile([P, NT], dt.float32, tag="rx")
            m_all = small.tile([P, NT], dt.float32, tag="m_all")

            # --- loads: w first (its global amax gates the whole w pipeline),
            # then x. 0.5MB per dma_start keeps per-tile completion receipts.
            # The last w and x tiles are split in column halves so the amax /
            # quant chains on the critical tail start half a tile earlier. ---
            for i in range(NK - 1):
                nc.sync.dma_start(w_sb[:, i, :], w_d[:, i, :])
            nc.sync.dma_start(w_sb[:, NK - 1, 0:512], w_d[:, NK - 1, 0:512])
            nc.sync.dma_start(w_sb[:, NK - 1, 512:1024], w_d[:, NK - 1, 512:1024])
            for i in range(NT - 1):
                nc.sync.dma_start(x_sb[:, i, :], x_d[:, i, :])
            nc.sync.dma_start(x_sb[:, NT - 1, 0:512], x_d[:, NT - 1, 0:512])
            nc.sync.dma_start(x_sb[:, NT - 1, 512:1024], x_d[:, NT - 1, 512:1024])

            # paired-transpose constant: idp[:, 0, 0:128] = I, idp[:, 1, 128:256] = I
            # (fp8: 1.0 is exactly representable).  One DoubleRow matmul with
            # lhsT = [xA | xB] then computes xA^T@[I|0] + xB^T@[0|I] = [xA^T | xB^T].
            id_f32 = const.tile([P, P], dt.float32, tag="id32f")
            make_identity(nc, id_f32[:])
            idp = const.tile([P, 2, 2 * P], dt.float8e4, tag="idp")
            nc.gpsimd.memset(idp[:], 0)
            nc.vector.tensor_copy(idp[:, 0, 0:P], id_f32[:])
            nc.vector.tensor_copy(idp[:, 1, P : 2 * P], id_f32[:])

            # --- w amax partials on vector, firing as each w tile lands
            # (the last w tile in halves to pull the global-amax gate in) ---
            for kk in range(NK - 1):
                nc.vector.reduce_max(
                    amw_parts[:, kk : kk + 1],
                    w_sb[:, kk, :],
                    axis=X,
                    apply_absolute_value=True,
                )
            nc.vector.reduce_max(
                amw_parts[:, NK - 1 : NK], w_sb[:, NK - 1, 0:512],
                axis=X, apply_absolute_value=True,
            )
            nc.vector.reduce_max(
                amw_parts[:, NK : NK + 1], w_sb[:, NK - 1, 512:1024],
                axis=X, apply_absolute_value=True,
            )
            nc.vector.reduce_max(amw_c[:], amw_parts[:], axis=X)
            nc.vector.tensor_scalar_max(amw_c[:], amw_c[:], EPS)
            nc.gpsimd.partition_all_reduce(
                amw_g[:], amw_c[:], channels=P, reduce_op=bass_isa.ReduceOp.max
            )
            # x0 amax on vector while gpsimd does the cross-partition hop
            nc.vector.reduce_max(
                amx_parts[:, 0:1], x_sb[:, 0, :], axis=X, apply_absolute_value=True
            )
            nc.vector.reciprocal(inv_w[:], amw_g[:])
            nc.vector.tensor_scalar_mul(rw[:], inv_w[:], E4M3_MAX / 2.0)
            nc.vector.tensor_scalar_mul(cw[:], amw_g[:], 4.0 / (E4M3_MAX * E4M3_MAX))

            def emit_x_head(tt):
                """amax (vector) for tile tt — skip tt=0 (emitted early above)."""
                if tt == 0:
                    return
                sl = slice(tt, tt + 1)
                if tt == NT - 1:
                    # split tile: partials into inv_x[:, tt] as scratch, merge
                    nc.vector.reduce_max(
                        amx_parts[:, sl], x_sb[:, tt, 0:512],
                        axis=X, apply_absolute_value=True,
                    )
                    nc.vector.reduce_max(
                        inv_x[:, sl], x_sb[:, tt, 512:1024],
                        axis=X, apply_absolute_value=True,
                    )
                    nc.vector.tensor_tensor(
                        amx_parts[:, sl], amx_parts[:, sl], inv_x[:, sl],
                        op=ALU.max,
                    )
                else:
                    nc.vector.reduce_max(
                        amx_parts[:, sl], x_sb[:, tt, :],
                        axis=X, apply_absolute_value=True,
                    )

            def emit_x_chain(tt):
                sl = slice(tt, tt + 1)
                nc.vector.tensor_scalar_max(amx_cl[:, sl], amx_parts[:, sl], EPS)
                nc.vector.reciprocal(inv_x[:, sl], amx_cl[:, sl])
                nc.vector.tensor_scalar_mul(rx[:, sl], inv_x[:, sl], E4M3_MAX / 2.0)
                # 1M-element quant on scalar (ACT); halves for the last tile
                if tt == NT - 1:
                    nc.scalar.mul(qx[:, tt, 0:512], x_sb[:, tt, 0:512], rx[:, sl])
                    nc.scalar.mul(qx[:, tt, 512:1024], x_sb[:, tt, 512:1024], rx[:, sl])
                else:
                    nc.scalar.mul(qx[:, tt, :], x_sb[:, tt, :], rx[:, sl])
                # paired PE transposes: jp covers kk = (2jp, 2jp+1)
                pxf = pt.tile([P, NK // 2, 2 * P], dt.float32, tag="pt")
                for jp in range(NK // 2):
                    lhsT = qx[:, tt, 2 * P * jp : 2 * P * (jp + 1)].rearrange(
                        "p (two f) -> p two f", two=2
                    )
                    nc.tensor.matmul(
                        pxf[:, jp, :], lhsT=lhsT, rhs=idp[:],
                        start=True, stop=True, perf_mode=DR,
                    )
                # evict PSUM -> qxT (contiguous [NK, P] per tt); gpsimd
                # cannot read PSUM, so alternate the two engines that can.
                # Last tile in halves so main j-pairs start on half 0.
                if tt == NT - 1:
                    nc.vector.tensor_copy(qxT[:, tt, 0 : NK // 2, :], pxf[:, 0 : NK // 4, :])
                    nc.scalar.copy(qxT[:, tt, NK // 2 : NK, :], pxf[:, NK // 4 : NK // 2, :])
                elif tt % 2 == 0:
                    nc.scalar.copy(qxT[:, tt, :, :], pxf[:])
                else:
                    nc.vector.tensor_copy(qxT[:, tt, :, :], pxf[:])

            def emit_main(tt):
                po = pm.tile([P, DOUT], dt.float32, tag="pm")
                for j in range(NK // 2):
                    st, sp = j == 0, j == NK // 2 - 1
                    for h in range(2):
                        nc.tensor.matmul(
                            po[:, h * 512 : (h + 1) * 512],
                            lhsT=qxT[:, tt, 2 * j : 2 * j + 2, :],
                            rhs=qwT[:, 2 * j : 2 * j + 2, h * 512 : (h + 1) * 512],
                            start=st, stop=sp, perf_mode=DR,
                        )
                sl = slice(tt, tt + 1)
                nc.vector.tensor_scalar(
                    m_all[:, sl], amx_cl[:, sl], cw[:], None, op0=ALU.mult
                )
                # scale+store in dout halves on both PSUM-capable engines:
                # halves the serial latency on the critical last tile
                ob = outp.tile([P, DOUT], dt.bfloat16, tag="ob")
                nc.vector.tensor_scalar_mul(ob[:, 0:512], po[:, 0:512], m_all[:, sl])
                nc.scalar.mul(ob[:, 512:1024], po[:, 512:1024], m_all[:, sl])
                nc.sync.dma_start(o_d[:, tt, :], ob[:])

            # --- per-token-tile pipeline.  qwT production is interleaved
            # tile-by-tile (scalar/vector split) so the first kk pairs exist
            # right when the main matmul's kk sweep reaches them; PE runs one
            # tile of transposes ahead of the main matmul. ---
            # emit_main(tt) reads every qwT pair, so mains trail the qwT
            # pair production (spread over the first four tile blocks) by
            # three blocks — emission order must respect write-before-read
            # for the Tile dependency tracker.
            for tt in range(NT):
                emit_x_head(tt)
                if tt < NK // 2:
                    nc.scalar.mul(qwT[:, 2 * tt, :], w_sb[:, 2 * tt, :], rw[:])
                    nc.vector.tensor_scalar_mul(
                        qwT[:, 2 * tt + 1, :], w_sb[:, 2 * tt + 1, :], rw[:]
                    )
                emit_x_chain(tt)
                if tt >= 3:
                    emit_main(tt - 3)
            for tt in range(NT - 3, NT):
                emit_main(tt)

    nc.compile()
    return nc


def get_nc():
    if "nc" not in _CACHE:
        _CACHE["nc"] = _build_nc()
    return _CACHE["nc"]


def make_in_maps(x, weight):
    x = np.ascontiguousarray(np.asarray(x, dtype=np.float32))
    w = np.ascontiguousarray(np.asarray(weight, dtype=np.float32))
    return [
        {
            "x": x[TPE * e : TPE * (e + 1)],
            "wt": np.ascontiguousarray(w[DOUT * e : DOUT * (e + 1)].T),
        }
        for e in range(NE)
    ]


def _host_reference(x, weight, tokens_per_expert):
    """Exact numpy port of the reference — fallback for non-uniform routing."""
    x = np.asarray(x, dtype=np.float32)
    w = np.asarray(weight, dtype=np.float32)
    tpe = np.asarray(tokens_per_expert, dtype=np.int64)
    ne = tpe.shape[0]
    T, din = x.shape
    dout = w.shape[0] // ne
    wr = w.reshape(ne, dout, din)

    def qd(v, axis, fmax):
        amax = np.max(np.abs(v), axis=axis, keepdims=True)
        scale = np.maximum(amax, EPS) / fmax
        q = np.clip(v / scale, -fmax, fmax).astype(ml_dtypes.float8_e4m3fn)
        return q.astype(np.float32) * scale

    w_dq = qd(wr, (1, 2), E4M3_MAX)
    x_dq = qd(x, -1, E4M3_MAX)
    offs = np.cumsum(tpe)
    starts = offs - tpe
    out = np.zeros((T, dout), np.float32)
    for e in range(ne):
        s, t = int(starts[e]), int(offs[e])
        if t > s:
            out[s:t] = x_dq[s:t] @ w_dq[e].T
    return out.astype(ml_dtypes.bfloat16)


def kernel(x, weight, tokens_per_expert):
    x = np.asarray(x)
    weight = np.asarray(weight)
    tpe = np.asarray(tokens_per_expert)
    uniform = (
        x.shape == (NE * TPE, DIN)
        and weight.shape == (NE * DOUT, DIN)
        and tpe.shape == (NE,)
        and bool(np.all(tpe.astype(np.int64) == TPE))
    )
    if not uniform:
        return _host_reference(x, weight, tpe)

    from concourse.bass_utils import run_bass_kernel_spmd

    nc = get_nc()
    in_maps = make_in_maps(x, weight)
    try:
        res = run_bass_kernel_spmd(nc, in_maps, core_ids=list(range(NE)))
    except Exception:
        # rare device wedge (NRT_EXEC_UNIT_UNRECOVERABLE) — reset and retry
        _axon_device_reset()
        res = run_bass_kernel_spmd(nc, in_maps, core_ids=list(range(NE)))
    return np.concatenate([res.results[e]["o"] for e in range(NE)], axis=0)


if __name__ == "__main__":
    rng = np.random.default_rng(0)
    x = rng.standard_normal((NE * TPE, DIN), dtype=np.float32)
    w = (rng.standard_normal((NE * DOUT, DIN), dtype=np.float32) * 0.02).astype(
        np.float32
    )
    tpe = np.full((NE,), TPE, dtype=np.int32)
    out = kernel(x, w, tpe)
    exp = _host_reference(x, w, tpe)
    a = out.astype(np.float64)
    b = exp.astype(np.float64)
    denom = max(np.abs(b).max(), 1e-30)
    print("absmax rel err:", np.abs(a - b).max() / denom)
    rms = np.sqrt(((a - b) ** 2).mean()) / np.sqrt((b**2).mean())
    print("rms rel err:", rms)
